# revision 4
# baseline (speedup 1.0000x reference)
"""Trainium2 Bass kernel for nn_NewRelativeEncoderLayer.

Reference computation (per batch b):
    q = x @ Wq.T ; k = x @ Wk.T ; v = x @ Wv.T          (heads split from 512)
    a_k = (pos @ rp_w1.T + rp_b1) @ rp_w2.T + rp_b2     (folded: pos @ Wt.T + ct)
    logits[h,j,i] = sum_d q[h,i,d] k[h,j,d] a_k[j,i,d] / T
    attn = softmax_i(logits) ; out = attn @ v  -> fc -> +res -> LN1 -> FFN -> LN2

Key algebraic trick: a_k[j,i,d] = sum_p pos[j,i,p]*Wt[d,p] + ct[d], so
    logits = sum_p pos[:,:,p] * G_p + G_c,   G_p[j,i] = sum_d k[j,d]*(q[i,d]*Wt[d,p]/T)
i.e. five standard K @ Qt^T matmuls combined elementwise with pos_mat.

Sharding (8 cores, no cross-core communication):
    core c: batch b = c//2, output-position slice j0 = (c%2)*256 (256 of 512 rows).
    Scores/softmax/attn@v/fc/FFN all shard cleanly by output position j.
"""

import sys

if "/opt/trn_rl_repo" not in sys.path:
    sys.path.insert(0, "/opt/trn_rl_repo")

import numpy as np

import concourse.bass as bass
import concourse.mybir as mybir
import concourse.tile as tile
from concourse import bacc
from concourse.bass_utils import run_bass_kernel_spmd
from concourse.masks import make_identity

F32 = mybir.dt.float32
AX = mybir.AxisListType
ALU = mybir.AluOpType
ACTF = mybir.ActivationFunctionType

B, N, DM, H, DK, DV, DP, DI = 4, 512, 512, 8, 64, 64, 4, 2048
TEMP = DK ** 0.5
NJ = 256          # j rows per core
NCORES = 8
LN_EPS = 1e-6


def _build(has_c: bool):
    nc = bacc.Bacc("TRN2", target_bir_lowering=False, debug=False)

    # ---- DRAM I/O (per core) ----
    enc = nc.dram_tensor("enc", [N, DM], F32, kind="ExternalInput")          # full x (batch b)
    encj = nc.dram_tensor("encj", [NJ, DM], F32, kind="ExternalInput")       # x[j0:j0+NJ]
    pos = nc.dram_tensor("pos", [NJ, N, DP], F32, kind="ExternalInput")      # pos_mat[b, j-slice]
    wq_t = nc.dram_tensor("wq_t", [DM, DM], F32, kind="ExternalInput")       # w_qs.T  [m, dq]
    wk_t = nc.dram_tensor("wk_t", [DM, DM], F32, kind="ExternalInput")       # w_ks.T  [m, dk]
    wv_t = nc.dram_tensor("wv_t", [DM, DM], F32, kind="ExternalInput")       # w_vs.T  [m, dv]
    wfc_t = nc.dram_tensor("wfc_t", [DM, DM], F32, kind="ExternalInput")     # w_fc.T  [hd, dm]
    w1_t = nc.dram_tensor("w1_t", [DM, DI], F32, kind="ExternalInput")       # ffn_w1.T [dm, di]
    w2_t = nc.dram_tensor("w2_t", [DI, DM], F32, kind="ExternalInput")       # ffn_w2.T [di, dm]
    sc = nc.dram_tensor("sc", [5, DM], F32, kind="ExternalInput")            # q-scale per channel
    g1 = nc.dram_tensor("g1", [DM], F32, kind="ExternalInput")
    b1 = nc.dram_tensor("b1", [DM], F32, kind="ExternalInput")
    g2 = nc.dram_tensor("g2", [DM], F32, kind="ExternalInput")
    b2 = nc.dram_tensor("b2", [DM], F32, kind="ExternalInput")
    fb1 = nc.dram_tensor("fb1", [DI], F32, kind="ExternalInput")
    fb2 = nc.dram_tensor("fb2", [DM], F32, kind="ExternalInput")
    attn_o = nc.dram_tensor("attn_o", [H, NJ, N], F32, kind="ExternalOutput")
    out_o = nc.dram_tensor("out_o", [NJ, DM], F32, kind="ExternalOutput")

    def bcast(ap_1d, p=128):
        a = ap_1d[:]
        return bass.AP(tensor=a.tensor, offset=a.offset, ap=[[0, p], *a.ap])

    with tile.TileContext(nc) as tc:
        import contextlib

        ctx = contextlib.ExitStack()
        with ctx:
            consts = ctx.enter_context(tc.tile_pool(name="consts", bufs=1))
            wts = ctx.enter_context(tc.tile_pool(name="wts", bufs=1))
            acts = ctx.enter_context(tc.tile_pool(name="acts", bufs=1))
            qtp = ctx.enter_context(tc.tile_pool(name="qtp", bufs=2))
            posp = ctx.enter_context(tc.tile_pool(name="posp", bufs=1))
            attp = ctx.enter_context(tc.tile_pool(name="attp", bufs=3))
            tmpp = ctx.enter_context(tc.tile_pool(name="tmpp", bufs=2))
            atp = ctx.enter_context(tc.tile_pool(name="atp", bufs=2))
            smol = ctx.enter_context(tc.tile_pool(name="smol", bufs=6))
            lnp = ctx.enter_context(tc.tile_pool(name="lnp", bufs=1))
            w1s = ctx.enter_context(tc.tile_pool(name="w1s", bufs=8))
            w2s = ctx.enter_context(tc.tile_pool(name="w2s", bufs=4))

            # ---- constants ----
            ident = consts.tile([128, 128], F32)
            make_identity(nc, ident)
            sc_sb = consts.tile([128, 5, 4], F32)
            nc.sync.dma_start(out=sc_sb, in_=sc[:].rearrange("s (c p) -> p s c", p=128))
            eps_sb = consts.tile([128, 1], F32)
            nc.vector.memset(eps_sb, LN_EPS)
            g1b = consts.tile([128, DM], F32)
            nc.gpsimd.dma_start(out=g1b, in_=bcast(g1))
            b1b = consts.tile([128, DM], F32)
            nc.gpsimd.dma_start(out=b1b, in_=bcast(b1))
            g2b = consts.tile([128, DM], F32)
            nc.gpsimd.dma_start(out=g2b, in_=bcast(g2))
            b2b = consts.tile([128, DM], F32)
            nc.gpsimd.dma_start(out=b2b, in_=bcast(b2))
            fb2b = consts.tile([128, DM], F32)
            nc.gpsimd.dma_start(out=fb2b, in_=bcast(fb2))
            fb1_sb = consts.tile([128, 16], F32)
            nc.sync.dma_start(out=fb1_sb, in_=fb1[:].rearrange("(c p) -> p c", p=128))

            # ---- weights / activations in SBUF ----
            wq_sb = [wts.tile([128, DM], F32, tag=f"wq{i}", name=f"wq{i}") for i in range(4)]
            wk_sb = [wts.tile([128, DM], F32, tag=f"wk{i}", name=f"wk{i}") for i in range(4)]
            wv_sb = [wts.tile([128, DM], F32, tag=f"wv{i}", name=f"wv{i}") for i in range(4)]
            wfc_sb = [wts.tile([128, DM], F32, tag=f"wfc{i}", name=f"wfc{i}") for i in range(4)]
            for i in range(4):
                s = slice(i * 128, (i + 1) * 128)
                nc.sync.dma_start(out=wq_sb[i], in_=wq_t[s, :])
                nc.sync.dma_start(out=wk_sb[i], in_=wk_t[s, :])
                nc.sync.dma_start(out=wv_sb[i], in_=wv_t[s, :])
                nc.sync.dma_start(out=wfc_sb[i], in_=wfc_t[s, :])

            enc_sb = [acts.tile([128, DM], F32, tag=f"enc{i}", name=f"enc{i}") for i in range(4)]
            for i in range(4):
                nc.sync.dma_start(out=enc_sb[i], in_=enc[i * 128:(i + 1) * 128, :])
            encj_sb = [acts.tile([128, DM], F32, tag=f"encj{i}", name=f"encj{i}") for i in range(2)]
            for i in range(2):
                nc.sync.dma_start(out=encj_sb[i], in_=encj[i * 128:(i + 1) * 128, :])
            pos_sb = [posp.tile([128, N * DP], F32, tag=f"pos{i}", name=f"pos{i}") for i in range(2)]
            for i in range(2):
                nc.sync.dma_start(out=pos_sb[i], in_=pos[i * 128:(i + 1) * 128])

            xT = [acts.tile([128, N], F32, tag=f"xT{i}", name=f"xT{i}") for i in range(4)]
            xjT = [acts.tile([128, NJ], F32, tag=f"xjT{i}", name=f"xjT{i}") for i in range(4)]
            QT = [acts.tile([128, N], F32, tag=f"QT{i}", name=f"QT{i}") for i in range(4)]
            V = [acts.tile([128, DM], F32, tag=f"V{i}", name=f"V{i}") for i in range(4)]
            KTp = [acts.tile([128, NJ], F32, tag=f"KTp{i}", name=f"KTp{i}") for i in range(H)]
            aoT = [acts.tile([128, NJ], F32, tag=f"aoT{i}", name=f"aoT{i}") for i in range(4)]

            # ================= phase 1+2: transposes + projections =============
            with tc.tile_pool(name="ps_a", bufs=1, space="PSUM") as ps_a:
                # x^T  (full, for Q and V) and xj^T (j-slice, for K)
                for tc4 in range(4):
                    for dmc in range(4):
                        pt = ps_a.tile([128, 128], F32, tag=f"tt{tc4 % 2}", name=f"tt{tc4 % 2}")
                        nc.tensor.transpose(pt, enc_sb[tc4][:, dmc * 128:(dmc + 1) * 128], ident)
                        nc.scalar.copy(out=xT[dmc][:, tc4 * 128:(tc4 + 1) * 128], in_=pt)
                for tj in range(2):
                    for dmc in range(4):
                        pt = ps_a.tile([128, 128], F32, tag=f"tt{tj % 2}", name=f"tt{tj % 2}")
                        nc.tensor.transpose(pt, encj_sb[tj][:, dmc * 128:(dmc + 1) * 128], ident)
                        nc.scalar.copy(out=xjT[dmc][:, tj * 128:(tj + 1) * 128], in_=pt)

                for i in range(H):
                    nc.vector.memset(KTp[i], 0.0)

                # Q^T[dq, i] = sum_m wq_t[m, dq] * xT[m, i]
                for dqc in range(4):
                    pq = ps_a.tile([128, N], F32, tag=f"mm{dqc % 2}", name=f"mm{dqc % 2}")
                    for mc in range(4):
                        nc.tensor.matmul(pq, wq_sb[mc][:, dqc * 128:(dqc + 1) * 128], xT[mc],
                                         start=(mc == 0), stop=(mc == 3))
                    nc.scalar.copy(out=QT[dqc], in_=pq)
                # K^T[dk, j] (zero-padded per head into 128-row tiles)
                for dkc in range(4):
                    pk = ps_a.tile([128, NJ], F32, tag=f"mm{dkc % 2}", name=f"mm{dkc % 2}")
                    for mc in range(4):
                        nc.tensor.matmul(pk, wk_sb[mc][:, dkc * 128:(dkc + 1) * 128], xjT[mc],
                                         start=(mc == 0), stop=(mc == 3))
                    nc.vector.tensor_copy(out=KTp[2 * dkc][0:64, :], in_=pk[0:64, :])
                    nc.vector.tensor_copy(out=KTp[2 * dkc + 1][64:128, :], in_=pk[64:128, :])
                # V[i, dv] = sum_m xT[m, i] * wv_t[m, dv]
                for ic in range(4):
                    pv = ps_a.tile([128, DM], F32, tag=f"mm{ic % 2}", name=f"mm{ic % 2}")
                    for mc in range(4):
                        nc.tensor.matmul(pv, xT[mc][:, ic * 128:(ic + 1) * 128], wv_sb[mc],
                                         start=(mc == 0), stop=(mc == 3))
                    nc.scalar.copy(out=V[ic], in_=pv)

            # ================= phase 3: attention ==============================
            nch = 5 if has_c else 4
            with tc.tile_pool(name="ps_b", bufs=1, space="PSUM") as ps_b, \
                 tc.tile_pool(name="ps_bt", bufs=2, space="PSUM") as ps_bt:
                for hc in range(4):
                    qt = []
                    for p in range(nch):
                        q = qtp.tile([128, N], F32, tag=f"qt{p}", name=f"qt{p}")
                        nc.vector.tensor_scalar_mul(q, QT[hc], sc_sb[:, p, hc:hc + 1])
                        qt.append(q)
                    av = ps_b.tile([128, NJ], F32, tag="av", name="av")
                    for hh in range(2):
                        h = 2 * hc + hh
                        attnT = [atp.tile([128, NJ], F32, tag=f"attnT{i}", name=f"attnT{i}") for i in range(4)]
                        for jc in range(2):
                            kslice = KTp[h][:, jc * 128:(jc + 1) * 128]
                            at = attp.tile([128, N], F32, tag="attn", name="attn")
                            if has_c:
                                gc = ps_b.tile([128, N], F32, tag="gc", name="gc")
                                nc.tensor.matmul(gc, kslice, qt[4], start=True, stop=True)
                                nc.scalar.copy(out=at, in_=gc)
                            g = [ps_b.tile([128, N], F32, tag=f"g{p}", name=f"g{p}") for p in range(4)]
                            for p in range(4):
                                nc.tensor.matmul(g[p], kslice, qt[p], start=True, stop=True)
                            pv3 = pos_sb[jc].rearrange("j (i p) -> j i p", p=DP)
                            t = [tmpp.tile([128, N], F32, tag=f"t{p}", name=f"t{p}") for p in range(4)]
                            for p in range(4):
                                nc.vector.tensor_tensor(t[p], g[p], pv3[:, :, p], ALU.mult)
                            nc.gpsimd.tensor_add(out=t[0], in0=t[0], in1=t[1])
                            nc.gpsimd.tensor_add(out=t[2], in0=t[2], in1=t[3])
                            if has_c:
                                nc.vector.tensor_add(out=t[0], in0=t[0], in1=t[2])
                                nc.vector.tensor_add(out=at, in0=at, in1=t[0])
                            else:
                                nc.vector.tensor_add(out=at, in0=t[0], in1=t[2])
                            # softmax over i (free dim); logits are O(5) so exp
                            # without max-subtraction is safe in fp32
                            ssum = smol.tile([128, 1], F32, tag="ssum", name="ssum")
                            nc.scalar.activation(out=at, in_=at, func=ACTF.Exp,
                                                 accum_out=ssum)
                            rinv = smol.tile([128, 1], F32, tag="rinv", name="rinv")
                            nc.vector.reciprocal(out=rinv, in_=ssum)
                            nc.scalar.mul(out=at, in_=at, mul=rinv)
                            nc.sync.dma_start(out=attn_o[h, jc * 128:(jc + 1) * 128, :], in_=at)
                            for ib in range(4):
                                pt = ps_bt.tile([128, 128], F32, tag="tt", name="tt")
                                nc.tensor.transpose(pt, at[:, ib * 128:(ib + 1) * 128], ident)
                                nc.scalar.copy(out=attnT[ib][:, jc * 128:(jc + 1) * 128], in_=pt)
                        # attn @ v  ->  aoT[hd, j]
                        for ic in range(4):
                            nc.tensor.matmul(av[hh * 64:(hh + 1) * 64, :],
                                             V[ic][:, h * 64:(h + 1) * 64], attnT[ic],
                                             start=(ic == 0), stop=(ic == 3))
                    nc.scalar.copy(out=aoT[hc], in_=av)

            # ================= phases 4-7: fc + LN1 + FFN + LN2 ================
            ln1_t = [lnp.tile([128, DM], F32, tag=f"ln1{i}", name=f"ln1{i}") for i in range(2)]
            lnT = [lnp.tile([128, NJ], F32, tag=f"lnT{i}", name=f"lnT{i}") for i in range(4)]
            hT = lnp.tile([128, 16, NJ], F32, tag="hT", name="hT")

            with tc.tile_pool(name="ps_c", bufs=2, space="PSUM") as ps_c:
                for tcc in range(2):
                    pfc = ps_c.tile([128, DM], F32, tag="mm", name="mm")
                    for hc4 in range(4):
                        nc.tensor.matmul(pfc, aoT[hc4][:, tcc * 128:(tcc + 1) * 128], wfc_sb[hc4],
                                         start=(hc4 == 0), stop=(hc4 == 3))
                    o1 = tmpp.tile([128, DM], F32, tag="o1", name="o1")
                    nc.vector.tensor_add(out=o1, in0=pfc, in1=encj_sb[tcc])
                    # LN1
                    st6 = smol.tile([128, 6], F32, tag="st6", name="st6")
                    nc.vector.bn_stats(out=st6, in_=o1)
                    mv = smol.tile([128, 2], F32, tag="mv", name="mv")
                    nc.vector.bn_aggr(out=mv, in_=st6)
                    sd = smol.tile([128, 1], F32, tag="sd", name="sd")
                    nc.scalar.activation(out=sd, in_=mv[:, 1:2], func=ACTF.Sqrt, bias=eps_sb)
                    rstd = smol.tile([128, 1], F32, tag="rstd", name="rstd")
                    nc.vector.reciprocal(out=rstd, in_=sd)
                    nc.vector.tensor_scalar(out=ln1_t[tcc], in0=o1,
                                            scalar1=mv[:, 0:1], scalar2=rstd,
                                            op0=ALU.subtract, op1=ALU.mult)
                    nc.vector.tensor_mul(out=ln1_t[tcc], in0=ln1_t[tcc], in1=g1b)
                    nc.vector.tensor_add(out=ln1_t[tcc], in0=ln1_t[tcc], in1=b1b)
                    for dmc in range(4):
                        pt = ps_c.tile([128, 128], F32, tag="tt", name="tt")
                        nc.tensor.transpose(pt, ln1_t[tcc][:, dmc * 128:(dmc + 1) * 128], ident)
                        nc.scalar.copy(out=lnT[dmc][:, tcc * 128:(tcc + 1) * 128], in_=pt)

                # FFN1: h^T[di, tok] = relu(w1_t^T @ lnT + b1)
                for dic in range(16):
                    ph = ps_c.tile([128, NJ], F32, tag="mm", name="mm")
                    for mc in range(4):
                        wtile = w1s.tile([128, 128], F32, tag="w1", name="w1")
                        nc.sync.dma_start(out=wtile,
                                          in_=w1_t[mc * 128:(mc + 1) * 128,
                                                   dic * 128:(dic + 1) * 128])
                        nc.tensor.matmul(ph, wtile, lnT[mc], start=(mc == 0), stop=(mc == 3))
                    nc.scalar.activation(out=hT[:, dic, :], in_=ph, func=ACTF.Relu,
                                         bias=fb1_sb[:, dic:dic + 1])
                # FFN2 + residual + LN2
                for tcc in range(2):
                    po = ps_c.tile([128, DM], F32, tag="mm", name="mm")
                    for dic in range(16):
                        wtile = w2s.tile([128, DM], F32, tag="w2", name="w2")
                        nc.sync.dma_start(out=wtile, in_=w2_t[dic * 128:(dic + 1) * 128, :])
                        nc.tensor.matmul(po, hT[:, dic, tcc * 128:(tcc + 1) * 128], wtile,
                                         start=(dic == 0), stop=(dic == 15))
                    o2 = tmpp.tile([128, DM], F32, tag="o2", name="o2")
                    nc.vector.tensor_add(out=o2, in0=po, in1=ln1_t[tcc])
                    nc.vector.tensor_add(out=o2, in0=o2, in1=fb2b)
                    st6 = smol.tile([128, 6], F32, tag="st6", name="st6")
                    nc.vector.bn_stats(out=st6, in_=o2)
                    mv = smol.tile([128, 2], F32, tag="mv", name="mv")
                    nc.vector.bn_aggr(out=mv, in_=st6)
                    sd = smol.tile([128, 1], F32, tag="sd", name="sd")
                    nc.scalar.activation(out=sd, in_=mv[:, 1:2], func=ACTF.Sqrt, bias=eps_sb)
                    rstd = smol.tile([128, 1], F32, tag="rstd", name="rstd")
                    nc.vector.reciprocal(out=rstd, in_=sd)
                    nc.vector.tensor_scalar(out=o2, in0=o2,
                                            scalar1=mv[:, 0:1], scalar2=rstd,
                                            op0=ALU.subtract, op1=ALU.mult)
                    nc.vector.tensor_mul(out=o2, in0=o2, in1=g2b)
                    nc.vector.tensor_add(out=o2, in0=o2, in1=b2b)
                    nc.sync.dma_start(out=out_o[tcc * 128:(tcc + 1) * 128, :], in_=o2)

    nc.compile()
    return nc


_BUILT = {}


def _get(has_c: bool):
    if has_c not in _BUILT:
        _BUILT[has_c] = _build(has_c)
    return _BUILT[has_c]


def _prepare(inputs):
    f = lambda k: np.ascontiguousarray(np.asarray(inputs[k], dtype=np.float32))
    enc_in = f("enc_input")
    pos_mat = f("pos_mat")
    Wt = (f("rp_w2") @ f("rp_w1")) / TEMP                 # [DK, DP]
    ct = (f("rp_w2") @ f("rp_b1") + f("rp_b2")) / TEMP    # [DK]
    has_c = bool(np.any(ct != 0.0))
    sc = np.zeros((5, DM), np.float32)
    sc[:4] = np.tile(Wt.T, (1, H))
    sc[4] = np.tile(ct, H)

    shared = {
        "wq_t": np.ascontiguousarray(f("w_qs").T),
        "wk_t": np.ascontiguousarray(f("w_ks").T),
        "wv_t": np.ascontiguousarray(f("w_vs").T),
        "wfc_t": np.ascontiguousarray(f("w_fc").T),
        "w1_t": np.ascontiguousarray(f("ffn_w1").T),
        "w2_t": np.ascontiguousarray(f("ffn_w2").T),
        "sc": sc,
        "g1": f("ln1_g"), "b1": f("ln1_b"), "g2": f("ln2_g"), "b2": f("ln2_b"),
        "fb1": f("ffn_b1"), "fb2": f("ffn_b2"),
    }
    in_maps = []
    for c in range(NCORES):
        b, j0 = c // 2, (c % 2) * NJ
        m = dict(shared)
        m["enc"] = enc_in[b]
        m["encj"] = np.ascontiguousarray(enc_in[b, j0:j0 + NJ])
        m["pos"] = np.ascontiguousarray(pos_mat[b, j0:j0 + NJ])
        in_maps.append(m)
    return _get(has_c), in_maps


def _assemble(results):
    out2 = np.empty((B, N, DM), np.float32)
    attn = np.empty((B, H, N, N), np.float32)
    for c in range(NCORES):
        b, j0 = c // 2, (c % 2) * NJ
        out2[b, j0:j0 + NJ] = results[c]["out_o"]
        attn[b, :, j0:j0 + NJ, :] = results[c]["attn_o"]
    return out2, attn


def kernel(**inputs):
    nc, in_maps = _prepare(inputs)
    res = run_bass_kernel_spmd(nc, in_maps, list(range(NCORES)))
    return _assemble(res.results)


def kernel_traced(**inputs):
    """Like kernel() but also returns NTFF-profiled exec time (ns, max over cores)."""
    nc, in_maps = _prepare(inputs)
    res = run_bass_kernel_spmd(nc, in_maps, list(range(NCORES)), trace=True,
                               trace_cores=list(range(NCORES)))
    return _assemble(res.results), res
